# revision 1
# baseline (speedup 1.0000x reference)
"""GCN policy network (2x GCNConv + global max pool + linear head) on 8 TRN2
NeuronCores.

Nodes are split into 8 windows; each core aggregates messages for its own
window via per-(core, src-window) degree-bucketed ap_gather slots, with
segment sums as single strided tensor_reduce ops into a bf16 partial. The
bucket->canonical realign runs on local_scatter (Q7 local-RAM scatter,
~4cyc/idx vs ap_gather's ~34); columns are chunk-major (7 chunks of 1792
nodes) so each scatter reads a contiguous range. Slot padding is minimized
by (a) a per-core greedy node->chunk balancing permutation (the pooled max
pool output is permutation-invariant) and (b) pooled bucket capacities via
a cumulative-max construction, with overflow nodes spilled into higher-k
buckets padded by gathers of a dedicated zeroed table column. AllGathers
are split in two for collective/compute overlap; activations stream through
DRAM; message and scratch pools are double-buffered.
"""

import numpy as np

import concourse.bass as bass
import concourse.mybir as mybir
import concourse.bacc as bacc
import concourse.tile as tile
from concourse import bass_utils

F_IN = 128
H = 16
A = 10
N_CORES = 8
TCH = 1792  # node chunk for local_scatter realign (<= 2046, divides 12544)


def _balance_perm(edge_index: np.ndarray, n_nodes: int, nw: int):
    """Per-core node permutation: assign each core's nodes to the 7 realign
    chunks so per-(src-group, degree) counts are even across chunks. This
    deflates the shared per-(chunk, degree) bucket caps (maxed over all
    (core, group) pairs), cutting padded gather slots. Returns old_of_new
    [N_CORES, nw]: new local j holds the node formerly at old_of_new[c, j].
    """
    src = edge_index[0].astype(np.int64)
    dst = edge_index[1].astype(np.int64)
    nt = nw // TCH
    old_of_new = np.empty((N_CORES, nw), np.int64)
    for c in range(N_CORES):
        m = (dst // nw) == c
        dl = (dst[m] % nw).astype(np.int64)
        gl = (src[m] // nw).astype(np.int64)
        kmaxc = 32
        prof = np.zeros((nw, N_CORES), np.int32)
        np.add.at(prof, (dl, gl), 1)
        prof = np.minimum(prof, kmaxc - 1)
        order = np.argsort(-(prof.astype(np.int64) ** 2).sum(1),
                           kind="stable")
        counts = np.zeros((nt, N_CORES, kmaxc), np.int32)
        fill = np.zeros(nt, np.int64)
        assign = np.empty(nw, np.int8)
        gidx = np.arange(N_CORES)
        for n in order:
            p = prof[n]
            crowd = counts[:, gidx, p].sum(1)
            crowd[fill >= TCH] = 1 << 30
            t = int(np.argmin(crowd))
            assign[n] = t
            counts[t, gidx, p] += 1
            fill[t] += 1
        old_of_new[c] = np.argsort(assign, kind="stable")
    return old_of_new


def _plan(edge_index: np.ndarray, n_nodes: int, nw: int, chunk: int):
    src = edge_index[0].astype(np.int64)
    dst = edge_index[1].astype(np.int64)
    ntot = nw * N_CORES
    nt = nw // TCH

    deg_in = np.bincount(dst, minlength=ntot)[:ntot].astype(np.float64)

    core_of = dst // nw
    grp_of = src // nw
    dstloc = (dst % nw).astype(np.int32)
    srcloc = (src % nw).astype(np.int32)

    sub = {}
    for c in range(N_CORES):
        mc = core_of == c
        for g in range(N_CORES):
            m = mc & (grp_of == g)
            sub[(c, g)] = (srcloc[m], dstloc[m])

    kmap = {}
    kmax = 1
    for key, (s, d) in sub.items():
        cnt = (np.bincount(d, minlength=nw) if len(d)
               else np.zeros(nw, np.int64))
        kmap[key] = cnt
        kmax = max(kmax, int(cnt.max()) if len(d) else 1)

    # caps per (chunk t, degree k): max over (c,g) of count of chunk-t nodes
    # with degree k
    # pooled caps: cum[t,k] = max over pairs of #chunk-t nodes with deg >= k;
    # cap[t,k] = cum[k] - cum[k+1]. Nodes overflow to higher-k buckets with
    # zero-column padding (always feasible by the cumulative construction).
    cum = np.zeros((nt, kmax + 2), np.int64)
    for cnt in kmap.values():
        cc = cnt.reshape(nt, TCH)
        for t in range(nt):
            nz = cc[t][cc[t] > 0]
            if len(nz):
                h = np.bincount(nz, minlength=kmax + 2)
                cumv = np.cumsum(h[::-1])[::-1]
                cum[t] = np.maximum(cum[t], cumv)
    cap = np.zeros((nt, kmax + 1), np.int64)
    for t in range(nt):
        for k in range(1, kmax + 1):
            cap[t, k] = cum[t, k] - cum[t, k + 1]

    # chunk-major column layout; each chunk's range padded to even
    tstart = np.zeros(nt + 1, np.int64)
    bucket_cols = {}  # (t, k) -> col start
    col = 0
    for t in range(nt):
        tstart[t] = col
        for k in range(1, kmax + 1):
            if cap[t, k] > 0:
                bucket_cols[(t, k)] = col
                col += int(cap[t, k])
        col += col % 2  # keep chunk ranges even for local_scatter num_idxs
    tstart[nt] = col
    ncols = col
    assert ncols < 32767

    # slot plan shared by every (core, group)
    instrs = []  # (slot_off, n_idx, [(roff, m, k, t, coff), ...])
    cur, cur_len, slot_off = [], 0, 0
    for t in range(nt):
        for k in range(1, kmax + 1):
            n = int(cap[t, k])
            if n == 0:
                continue
            done = 0
            while done < n:
                m = min(n - done, max(1, chunk // k))
                span_len = m * k
                if cur_len + span_len > chunk and cur_len > 0:
                    pad = (-cur_len) % 32
                    instrs.append((slot_off, cur_len + pad, cur))
                    slot_off += cur_len + pad
                    cur, cur_len = [], 0
                cur.append((cur_len, m, k, t, bucket_cols[(t, k)] + done))
                cur_len += span_len
                done += m
    if cur_len:
        pad = (-cur_len) % 32
        instrs.append((slot_off, cur_len + pad, cur))
        slot_off += cur_len + pad
    total_slots = slot_off

    idx_all = np.zeros((N_CORES, N_CORES, total_slots), np.int16)
    scat_all = np.full((N_CORES, N_CORES, ncols), -1, np.int16)
    for c in range(N_CORES):
        for g in range(N_CORES):
            s, d = sub[(c, g)]
            cnt = kmap[(c, g)]
            if len(d) == 0:
                continue
            order = np.lexsort((s, d))
            s, d = s[order], d[order]
            dvals = np.unique(d)
            kofd = cnt[dvals]
            runstart = np.concatenate([[0], np.cumsum(kofd)[:-1]])
            tvals = dvals // TCH
            s_ext = np.concatenate([s, np.full(kmax + 1, nw, s.dtype)])
            for t in range(nt):
                # ascending spill-up placement: bucket k gets deg-k nodes
                # first, then highest-degree carried overflow (zero-padded)
                int_r = np.arange(len(dvals))
                placed = {}
                carry = []
                for k in range(1, kmax + 1):
                    new = int_r[(kofd == k) & (tvals == t)]
                    avail = list(new) + carry
                    ck = int(cap[t, k])
                    placed[k] = avail[:ck]
                    carry = avail[ck:]
                assert not carry
                for k in range(1, kmax + 1):
                    if cap[t, k] == 0 or not placed[k]:
                        continue
                    pk = np.array(placed[k], np.int64)
                    selnodes = dvals[pk]
                    nsel = len(selnodes)
                    pos = runstart[pk]
                    realk = kofd[pk]
                    coff0 = bucket_cols[(t, k)]
                    scat_all[c, g, coff0:coff0 + nsel] = (
                        selnodes - t * TCH).astype(np.int16)
                    for (soff, nidx, spans) in instrs:
                        for (roff, m, kk, tt, coff) in spans:
                            if kk != k or tt != t:
                                continue
                            lo = coff - coff0
                            hi = min(lo + m, nsel)
                            if hi <= lo:
                                continue
                            nodes = np.arange(lo, hi)
                            starts = soff + roff + (nodes - lo) * k
                            slotpos = (starts[:, None]
                                       + np.arange(k)[None, :])
                            ar = np.arange(k)[None, :]
                            valid = ar < realk[nodes][:, None]
                            srcpos = np.where(
                                valid, pos[nodes][:, None] + ar, len(s))
                            idx_all[c, g, slotpos.ravel()] = (
                                s_ext[srcpos.ravel()])

    def wrap(arr, c):
        L = arr.shape[-1]
        Lc = (L + 15) // 16 * 16
        out = np.zeros((128, Lc // 16), np.int16)
        for g in range(N_CORES):
            a = np.zeros(Lc, np.int16)
            a[:L] = arr[c, g]
            out[16 * g:16 * g + 16, :] = a.reshape(Lc // 16, 16).T
        return out

    idx_w = [wrap(idx_all, c) for c in range(N_CORES)]

    # local_scatter idxs: [128, ncols], per-partition independent; replicate
    # group g's stream on its 16 partitions
    sidx_w = []
    for c in range(N_CORES):
        o = np.zeros((128, ncols), np.int16)
        for g in range(N_CORES):
            o[16 * g:16 * g + 16, :] = scat_all[c, g][None, :]
        sidx_w.append(o)

    deg_full = np.full(ntot, 1e30, np.float32)
    deg_full[:n_nodes] = (deg_in[:n_nodes] + 1.0).astype(np.float32)

    return instrs, total_slots, ncols, tstart, idx_w, sidx_w, deg_full


def _build(nw, total_slots, ncols, tstart, instrs, chunk):
    nt = nw // TCH
    nc = bacc.Bacc("TRN2", target_bir_lowering=False, debug=False,
                   num_devices=N_CORES)
    dt = mybir.dt.float32
    bt = mybir.dt.bfloat16

    xT_in = nc.dram_tensor("xT", [F_IN, nw], dt, kind="ExternalInput")
    deg_in = nc.dram_tensor("degv", [1, nw], dt, kind="ExternalInput")
    ones_in = nc.dram_tensor("ones16", [1, 16], dt, kind="ExternalInput")
    ident_in = nc.dram_tensor("ident", [16, 16], dt, kind="ExternalInput")
    idx_in = nc.dram_tensor("idxs", [128, (total_slots + 15) // 16],
                            mybir.dt.int16, kind="ExternalInput")
    ridx_in = nc.dram_tensor("ridxs", [128, ncols], mybir.dt.int16,
                             kind="ExternalInput")
    blk_in = nc.dram_tensor("blk", [128, 16], dt, kind="ExternalInput")
    w1_in = nc.dram_tensor("W1", [F_IN, H], dt, kind="ExternalInput")
    b1_in = nc.dram_tensor("b1", [H, 1], dt, kind="ExternalInput")
    w2_in = nc.dram_tensor("W2", [H, H], dt, kind="ExternalInput")
    b2_in = nc.dram_tensor("b2", [H, 1], dt, kind="ExternalInput")
    wc_in = nc.dram_tensor("Wc", [H, A], dt, kind="ExternalInput")
    bc_in = nc.dram_tensor("bc", [1, A], dt, kind="ExternalInput")
    out_t = nc.dram_tensor("out", [1, A], dt, kind="ExternalOutput")

    with tile.TileContext(nc) as tc:
        with (
            tc.tile_pool(name="persist", bufs=1) as pp,
            tc.tile_pool(name="tabp", bufs=1) as tabp,
            tc.tile_pool(name="partp", bufs=1) as partp,
            tc.tile_pool(name="msgp", bufs=2) as msgp,
            tc.tile_pool(name="alnp", bufs=2) as alnp,
            tc.tile_pool(name="smallp", bufs=2) as smallp,
            tc.tile_pool(name="psum", bufs=2, space="PSUM") as psp,
            tc.tile_pool(name="dram", bufs=1, space="DRAM") as dram,
        ):
            w1 = pp.tile([F_IN, H], dt)
            nc.sync.dma_start(w1[:], w1_in[:])
            w2 = pp.tile([H, H], dt)
            nc.sync.dma_start(w2[:], w2_in[:])
            wc = pp.tile([H, A], dt)
            nc.sync.dma_start(wc[:], wc_in[:])
            b1 = pp.tile([H, 1], dt)
            nc.sync.dma_start(b1[:], b1_in[:])
            b2 = pp.tile([H, 1], dt)
            nc.sync.dma_start(b2[:], b2_in[:])
            bcb = pp.tile([1, A], dt)
            nc.sync.dma_start(bcb[:], bc_in[:])
            blk = pp.tile([128, 16], dt)
            nc.sync.dma_start(blk[:], blk_in[:])
            blkb = pp.tile([128, 16], bt)
            nc.vector.tensor_copy(blkb[:], blk[:])
            ones16 = pp.tile([1, 16], dt)
            nc.sync.dma_start(ones16[:], ones_in[:])
            idxs = pp.tile([128, (total_slots + 15) // 16], mybir.dt.int16)
            nc.sync.dma_start(idxs[:], idx_in[:])
            sidxs = pp.tile([128, ncols], mybir.dt.int16)
            nc.sync.dma_start(sidxs[:], ridx_in[:])

            h_dram = dram.tile([H, nw], dt, tag="hdram")
            pooled = smallp.tile([16, 1], dt, tag="pool")
            dis_dram = dram.tile([1, nw], dt, tag="disd")
            for j in range(0, nw, 512):
                cw = min(512, nw - j)
                degc = smallp.tile([1, 512], dt, tag="degc")
                nc.sync.dma_start(degc[:, :cw], deg_in[:, j:j + cw])
                nc.vector.reciprocal(degc[:, :cw], degc[:, :cw])
                disc = smallp.tile([1, 512], dt, tag="disc")
                nc.scalar.activation(disc[:, :cw], degc[:, :cw],
                                     mybir.ActivationFunctionType.Sqrt)
                nc.sync.dma_start(dis_dram[:, j:j + cw], disc[:, :cw])

            for layer in range(2):
                w = w1 if layer == 0 else w2
                bvec = b1 if layer == 0 else b2
                kdim = F_IN if layer == 0 else H

                nwA = 3 * TCH  # AllGather split point (chunk-aligned)
                ag_inA = dram.tile([16, nwA], dt, tag=f"aginA{layer}")
                ag_inB = dram.tile([16, nw - nwA], dt, tag=f"aginB{layer}")
                ag_outA = dram.tile([128, nwA], dt, tag=f"agoutA{layer}")
                ag_outB = dram.tile([128, nw - nwA], dt, tag=f"agoutB{layer}")

                sb = nc.enter_named_scope(f"sbuild{layer}", False)
                for j in range(0, nw, 512):
                    cw = min(512, nw - j)
                    if layer == 0:
                        xc = smallp.tile([F_IN, 512], dt, tag="xc")
                        nc.sync.dma_start(xc[:, :cw], xT_in[:, j:j + cw])
                        rhs_ap = xc[:kdim, :cw]
                    else:
                        hc = smallp.tile([H, 512], dt, tag="hc")
                        nc.sync.dma_start(hc[:, :cw], h_dram[:, j:j + cw])
                        rhs_ap = hc[:kdim, :cw]
                    ps = psp.tile([16, 512], dt, tag="mma")
                    nc.tensor.matmul(ps[:, :cw], lhsT=w[:kdim, :], rhs=rhs_ap,
                                     start=True, stop=True)
                    disc = smallp.tile([1, 512], dt, tag="disc")
                    nc.sync.dma_start(disc[:, :cw], dis_dram[:, j:j + cw])
                    psd = psp.tile([16, 512], dt, tag="mmd")
                    nc.tensor.matmul(psd[:, :cw], lhsT=ones16[:],
                                     rhs=disc[:, :cw], start=True, stop=True)
                    ts = smallp.tile([16, 512], dt, tag="sc")
                    nc.vector.tensor_copy(ts[:, :cw], ps[:, :cw])
                    sc = smallp.tile([16, 512], dt, tag="sct")
                    nc.vector.tensor_mul(sc[:, :cw], ts[:, :cw], psd[:, :cw])
                    if j + cw <= nwA:
                        nc.sync.dma_start(ag_inA[:, j:j + cw], sc[:, :cw])
                    elif j >= nwA:
                        nc.sync.dma_start(ag_inB[:, j - nwA:j - nwA + cw],
                                          sc[:, :cw])
                    else:
                        cA = nwA - j
                        nc.sync.dma_start(ag_inA[:, j:nwA], sc[:, :cA])
                        nc.sync.dma_start(ag_inB[:, 0:j + cw - nwA],
                                          sc[:, cA:cw])

                nc.leave_named_scope(f"sbuild{layer}", sb[0], False)
                agc = nc.enter_named_scope(f"ag{layer}", False)
                nc.gpsimd.collective_compute(
                    "AllGather", mybir.AluOpType.bypass,
                    replica_groups=[list(range(N_CORES))],
                    ins=[ag_inA.opt()], outs=[ag_outA.opt()],
                )
                nc.gpsimd.collective_compute(
                    "AllGather", mybir.AluOpType.bypass,
                    replica_groups=[list(range(N_CORES))],
                    ins=[ag_inB.opt()], outs=[ag_outB.opt()],
                )
                nc.leave_named_scope(f"ag{layer}", agc[0], False)

                table = tabp.tile([128, nw + 1], dt, tag="table")
                nc.sync.dma_start(table[:, :nwA], ag_outA[:])
                nc.sync.dma_start(table[:, nwA:nw], ag_outB[:])
                nc.vector.memset(table[:, nw:nw + 1], 0.0)

                partial = partp.tile([128, ncols], bt, tag="partial")
                nc.vector.memset(partial[:], 0.0)

                gsc = nc.enter_named_scope(f"gather{layer}", False)
                for (soff, nidx, spans) in instrs:
                    msg = msgp.tile([128, chunk], dt, tag="msg")
                    nc.gpsimd.ap_gather(
                        out_ap=msg[:, :nidx],
                        in_ap=table[:],
                        idxs_ap=idxs[:, soff // 16:(soff + nidx) // 16],
                        channels=128, num_elems=nw + 1, d=1, num_idxs=nidx,
                    )
                    for (roff, m, k, t, coff) in spans:
                        pslc = partial[:, coff:coff + m]
                        if k == 1:
                            nc.vector.tensor_copy(
                                pslc, msg[:, roff:roff + m])
                        else:
                            mv = msg[:, roff:roff + m * k].rearrange(
                                "p (n k) -> p n k", k=k)
                            with nc.allow_low_precision(
                                    reason="fp32 msgs reduced once to bf16 "
                                           "partial; rel tol 2e-2"):
                                nc.vector.tensor_reduce(
                                    pslc, mv, axis=mybir.AxisListType.X,
                                    op=mybir.AluOpType.add)

                nc.leave_named_scope(f"gather{layer}", gsc[0], False)
                rsc = nc.enter_named_scope(f"realign{layer}", False)
                for t in range(nt):
                    j0 = t * TCH
                    c0, c1 = int(tstart[t]), int(tstart[t + 1])
                    aln = alnp.tile([128, TCH], bt, tag="aln")
                    nc.gpsimd.local_scatter(
                        aln[:], partial[:, c0:c1], sidxs[:, c0:c1],
                        channels=128, num_elems=TCH, num_idxs=c1 - c0,
                    )
                    sob = alnp.tile([16, TCH], dt, tag="sob")
                    if j0 + TCH <= nwA:
                        nc.sync.dma_start(sob[:], ag_inA[:, j0:j0 + TCH])
                    else:
                        nc.sync.dma_start(
                            sob[:], ag_inB[:, j0 - nwA:j0 - nwA + TCH])
                    for j1 in range(0, TCH, 512):
                        j = j0 + j1
                        cw = min(512, TCH - j1)
                        ps = psp.tile([16, 512], dt, tag="mma")
                        nc.tensor.matmul(ps[:, :cw], lhsT=blkb[:],
                                         rhs=aln[:, j1:j1 + cw],
                                         start=True, stop=True)
                        disc = smallp.tile([1, 512], dt, tag="disc")
                        nc.sync.dma_start(disc[:, :cw], dis_dram[:, j:j + cw])
                        psd = psp.tile([16, 512], dt, tag="mmd")
                        nc.tensor.matmul(psd[:, :cw], lhsT=ones16[:],
                                         rhs=disc[:, :cw],
                                         start=True, stop=True)
                        u = smallp.tile([16, 512], dt, tag="acc")
                        nc.vector.tensor_add(u[:, :cw], sob[:, j1:j1 + cw],
                                             ps[:, :cw])
                        v = smallp.tile([16, 512], dt, tag="tso")
                        nc.vector.tensor_mul(v[:, :cw], u[:, :cw], psd[:, :cw])
                        hn = smallp.tile([16, 512], dt, tag="hn")
                        nc.scalar.activation(hn[:, :cw], v[:, :cw],
                                             mybir.ActivationFunctionType.Relu,
                                             bias=bvec[:])
                        if layer == 0:
                            nc.sync.dma_start(h_dram[:, j:j + cw], hn[:, :cw])
                        else:
                            pmx = smallp.tile([16, 1], dt, tag="pmx")
                            nc.vector.tensor_reduce(
                                pmx[:], hn[:, :cw],
                                axis=mybir.AxisListType.X,
                                op=mybir.AluOpType.max)
                            if j == 0:
                                nc.vector.tensor_copy(pooled[:], pmx[:])
                            else:
                                nc.vector.tensor_tensor(
                                    pooled[:], pooled[:], pmx[:],
                                    op=mybir.AluOpType.max)
                nc.leave_named_scope(f"realign{layer}", rsc[0], False)

            pin = dram.tile([16, 1], dt, tag="pin")
            pout = dram.tile([16, 1], dt, tag="pout")
            nc.sync.dma_start(pin[:], pooled[:])
            nc.gpsimd.collective_compute(
                "AllReduce", mybir.AluOpType.max,
                replica_groups=[list(range(N_CORES))],
                ins=[pin.opt()], outs=[pout.opt()],
            )
            pooled2 = smallp.tile([16, 1], dt, tag="pool2")
            nc.sync.dma_start(pooled2[:], pout[:])
            ps = psp.tile([1, A], dt, tag="mmc")
            nc.tensor.matmul(ps[:], lhsT=pooled2[:], rhs=wc[:],
                             start=True, stop=True)
            ores = smallp.tile([1, A], dt, tag="ores")
            nc.vector.tensor_add(ores[:], ps[:], bcb[:])
            nc.sync.dma_start(out_t[:], ores[:])

    nc.compile()
    return nc


def _prepare(x, edge_index, W1, b1, W2, b2, Wc, bc, _nw=12544, _chunk=2048):
    n_nodes = x.shape[0]
    nw = _nw
    edge_index = np.asarray(edge_index)

    # chunk-balancing node relabel (pool output is permutation-invariant)
    old_of_new = _balance_perm(edge_index, n_nodes, nw)  # [N_CORES, nw]
    new_of_old = np.empty_like(old_of_new)
    for c in range(N_CORES):
        new_of_old[c, old_of_new[c]] = np.arange(nw)

    def relab(a):
        cc = a // nw
        return cc * nw + new_of_old[cc, a % nw]

    edge2 = np.stack([relab(edge_index[0].astype(np.int64)),
                      relab(edge_index[1].astype(np.int64))])

    instrs, total_slots, ncols, tstart, idx_w, sidx_w, _ = _plan(
        edge2, n_nodes, nw, _chunk)

    # degrees/dis in relabeled space; pad nodes (old id >= real count) inert
    deg_new = np.bincount(edge2[1], minlength=nw * N_CORES).astype(np.float64)
    deg_full = np.full(nw * N_CORES, 1e30, np.float32)
    for c in range(N_CORES):
        rc = min(max(n_nodes - c * nw, 0), nw)
        rn = old_of_new[c] < rc
        seg = deg_full[c * nw:(c + 1) * nw]
        seg[rn] = (deg_new[c * nw:(c + 1) * nw][rn] + 1.0).astype(np.float32)

    nc = _build(nw, total_slots, ncols, tstart, instrs, _chunk)

    x = np.asarray(x, np.float32)
    blk = np.zeros((128, 16), np.float32)
    blk[np.arange(128), np.arange(128) % 16] = 1.0

    in_maps = []
    for c in range(N_CORES):
        xw = np.zeros((nw, F_IN), np.float32)
        rc = min(max(n_nodes - c * nw, 0), nw)
        rn = old_of_new[c] < rc
        if rc > 0:
            xw[rn] = x[c * nw + old_of_new[c][rn]]
        in_maps.append({
            "xT": np.ascontiguousarray(xw.T),
            "degv": deg_full[c * nw:(c + 1) * nw][None, :].copy(),
            "ones16": np.ones((1, 16), np.float32),
            "idxs": idx_w[c],
            "ridxs": sidx_w[c],
            "blk": blk,
            "ident": np.eye(16, dtype=np.float32),
            "W1": np.asarray(W1, np.float32),
            "b1": np.asarray(b1, np.float32).reshape(H, 1),
            "W2": np.asarray(W2, np.float32),
            "b2": np.asarray(b2, np.float32).reshape(H, 1),
            "Wc": np.asarray(Wc, np.float32),
            "bc": np.asarray(bc, np.float32).reshape(1, A),
        })
    return nc, in_maps


def kernel(x, edge_index, W1, b1, W2, b2, Wc, bc, _nw=12544, _chunk=2048,
           _run=None):
    nc, in_maps = _prepare(x, edge_index, W1, b1, W2, b2, Wc, bc,
                           _nw=_nw, _chunk=_chunk)

    if _run == "sim":
        from concourse.bass_interp import MultiCoreSim
        sim = MultiCoreSim(nc, num_cores=N_CORES, trace=False)
        for c in range(N_CORES):
            for k, v in in_maps[c].items():
                sim.cores[c].tensor(k)[:] = v
        sim.simulate()
        print("SIM time:", sim.global_time, "ns")
        return np.asarray(sim.cores[0].tensor("out")).reshape(A)

    res = bass_utils.run_bass_kernel_spmd(
        nc, in_maps, core_ids=list(range(N_CORES)),
        trace=bool(_run == "trace"))
    if _run == "trace":
        print("HW exec time:", res.exec_time_ns, "ns")
    return np.asarray(res.results[0]["out"]).reshape(A)



# revision 9
# speedup vs baseline: 1.0291x; 1.0291x over previous
"""GCN policy network (2x GCNConv + global max pool + linear head) on 8 TRN2
NeuronCores.

Nodes are split into 8 windows; each core aggregates messages for its own
window via per-(core, src-window) degree-bucketed ap_gather slots, with
segment sums as single strided tensor_reduce ops into a bf16 partial. The
bucket->canonical realign runs on local_scatter (Q7 local-RAM scatter,
~4cyc/idx vs ap_gather's ~34); columns are chunk-major (7 chunks of 1792
nodes) so each scatter reads a contiguous range. Slot padding is minimized
by (a) a per-core greedy node->chunk balancing permutation (the pooled max
pool output is permutation-invariant) and (b) pooled bucket capacities via
a cumulative-max construction, with overflow nodes spilled into higher-k
buckets padded by gathers of a dedicated zeroed table column. AllGathers
are split in two for collective/compute overlap; activations stream through
DRAM; message and scratch pools are double-buffered.
"""

import numpy as np

import concourse.bass as bass
import concourse.mybir as mybir
import concourse.bacc as bacc
import concourse.tile as tile
from concourse import bass_utils

F_IN = 128
H = 16
A = 10
N_CORES = 8
TCH = 1792  # node chunk for local_scatter realign (<= 2046, divides 12544)


def _balance_perm(edge_index: np.ndarray, n_nodes: int, nw: int):
    """Per-core node permutation: assign each core's nodes to the 7 realign
    chunks so per-(src-group, degree) counts are even across chunks. This
    deflates the shared per-(chunk, degree) bucket caps (maxed over all
    (core, group) pairs), cutting padded gather slots. Returns old_of_new
    [N_CORES, nw]: new local j holds the node formerly at old_of_new[c, j].
    """
    src = edge_index[0].astype(np.int64)
    dst = edge_index[1].astype(np.int64)
    nt = nw // TCH
    old_of_new = np.empty((N_CORES, nw), np.int64)
    for c in range(N_CORES):
        m = (dst // nw) == c
        dl = (dst[m] % nw).astype(np.int64)
        gl = (src[m] // nw).astype(np.int64)
        kmaxc = 32
        prof = np.zeros((nw, N_CORES), np.int32)
        np.add.at(prof, (dl, gl), 1)
        prof = np.minimum(prof, kmaxc - 1)
        order = np.argsort(-(prof.astype(np.int64) ** 2).sum(1),
                           kind="stable")
        counts = np.zeros((nt, N_CORES, kmaxc), np.int32)
        fill = np.zeros(nt, np.int64)
        assign = np.empty(nw, np.int8)
        gidx = np.arange(N_CORES)
        for n in order:
            p = prof[n]
            crowd = counts[:, gidx, p].sum(1)
            crowd[fill >= TCH] = 1 << 30
            t = int(np.argmin(crowd))
            assign[n] = t
            counts[t, gidx, p] += 1
            fill[t] += 1
        old_of_new[c] = np.argsort(assign, kind="stable")
    return old_of_new


def _plan(edge_index: np.ndarray, n_nodes: int, nw: int, chunk: int):
    src = edge_index[0].astype(np.int64)
    dst = edge_index[1].astype(np.int64)
    ntot = nw * N_CORES
    nt = nw // TCH

    deg_in = np.bincount(dst, minlength=ntot)[:ntot].astype(np.float64)

    core_of = dst // nw
    grp_of = src // nw
    dstloc = (dst % nw).astype(np.int32)
    srcloc = (src % nw).astype(np.int32)

    sub = {}
    for c in range(N_CORES):
        mc = core_of == c
        for g in range(N_CORES):
            m = mc & (grp_of == g)
            sub[(c, g)] = (srcloc[m], dstloc[m])

    kmap = {}
    kmax = 1
    for key, (s, d) in sub.items():
        cnt = (np.bincount(d, minlength=nw) if len(d)
               else np.zeros(nw, np.int64))
        kmap[key] = cnt
        kmax = max(kmax, int(cnt.max()) if len(d) else 1)

    # caps per (chunk t, degree k): max over (c,g) of count of chunk-t nodes
    # with degree k
    # pooled caps: cum[t,k] = max over pairs of #chunk-t nodes with deg >= k;
    # cap[t,k] = cum[k] - cum[k+1]. Nodes overflow to higher-k buckets with
    # zero-column padding (always feasible by the cumulative construction).
    cum = np.zeros((nt, kmax + 2), np.int64)
    for cnt in kmap.values():
        cc = cnt.reshape(nt, TCH)
        for t in range(nt):
            nz = cc[t][cc[t] > 0]
            if len(nz):
                h = np.bincount(nz, minlength=kmax + 2)
                cumv = np.cumsum(h[::-1])[::-1]
                cum[t] = np.maximum(cum[t], cumv)
    cap = np.zeros((nt, kmax + 1), np.int64)
    for t in range(nt):
        for k in range(1, kmax + 1):
            cap[t, k] = cum[t, k] - cum[t, k + 1]

    # chunk-major column layout; each chunk's range padded to even
    tstart = np.zeros(nt + 1, np.int64)
    bucket_cols = {}  # (t, k) -> col start
    col = 0
    for t in range(nt):
        tstart[t] = col
        for k in range(1, kmax + 1):
            if cap[t, k] > 0:
                bucket_cols[(t, k)] = col
                col += int(cap[t, k])
        col += col % 2  # keep chunk ranges even for local_scatter num_idxs
    tstart[nt] = col
    ncols = col
    assert ncols < 32767

    # slot plan shared by every (core, group)
    instrs = []  # (slot_off, n_idx, [(roff, m, k, t, coff), ...])
    cur, cur_len, slot_off = [], 0, 0
    for t in range(nt):
        for k in range(1, kmax + 1):
            n = int(cap[t, k])
            if n == 0:
                continue
            done = 0
            while done < n:
                m = min(n - done, max(1, chunk // k))
                span_len = m * k
                if cur_len + span_len > chunk and cur_len > 0:
                    pad = (-cur_len) % 32
                    instrs.append((slot_off, cur_len + pad, cur))
                    slot_off += cur_len + pad
                    cur, cur_len = [], 0
                cur.append((cur_len, m, k, t, bucket_cols[(t, k)] + done))
                cur_len += span_len
                done += m
    if cur_len:
        pad = (-cur_len) % 32
        instrs.append((slot_off, cur_len + pad, cur))
        slot_off += cur_len + pad
    total_slots = slot_off

    idx_all = np.zeros((N_CORES, N_CORES, total_slots), np.int16)
    scat_all = np.full((N_CORES, N_CORES, ncols), -1, np.int16)
    for c in range(N_CORES):
        for g in range(N_CORES):
            s, d = sub[(c, g)]
            cnt = kmap[(c, g)]
            if len(d) == 0:
                continue
            order = np.lexsort((s, d))
            s, d = s[order], d[order]
            dvals = np.unique(d)
            kofd = cnt[dvals]
            runstart = np.concatenate([[0], np.cumsum(kofd)[:-1]])
            tvals = dvals // TCH
            s_ext = np.concatenate([s, np.full(kmax + 1, nw, s.dtype)])
            for t in range(nt):
                # ascending spill-up placement: bucket k gets deg-k nodes
                # first, then highest-degree carried overflow (zero-padded)
                int_r = np.arange(len(dvals))
                placed = {}
                carry = []
                for k in range(1, kmax + 1):
                    new = int_r[(kofd == k) & (tvals == t)]
                    avail = list(new) + carry
                    ck = int(cap[t, k])
                    placed[k] = avail[:ck]
                    carry = avail[ck:]
                assert not carry
                for k in range(1, kmax + 1):
                    if cap[t, k] == 0 or not placed[k]:
                        continue
                    pk = np.array(placed[k], np.int64)
                    selnodes = dvals[pk]
                    nsel = len(selnodes)
                    pos = runstart[pk]
                    realk = kofd[pk]
                    coff0 = bucket_cols[(t, k)]
                    scat_all[c, g, coff0:coff0 + nsel] = (
                        selnodes - t * TCH).astype(np.int16)
                    for (soff, nidx, spans) in instrs:
                        for (roff, m, kk, tt, coff) in spans:
                            if kk != k or tt != t:
                                continue
                            lo = coff - coff0
                            hi = min(lo + m, nsel)
                            if hi <= lo:
                                continue
                            nodes = np.arange(lo, hi)
                            starts = soff + roff + (nodes - lo) * k
                            slotpos = (starts[:, None]
                                       + np.arange(k)[None, :])
                            ar = np.arange(k)[None, :]
                            valid = ar < realk[nodes][:, None]
                            srcpos = np.where(
                                valid, pos[nodes][:, None] + ar, len(s))
                            idx_all[c, g, slotpos.ravel()] = (
                                s_ext[srcpos.ravel()])

    def wrap(arr, c):
        L = arr.shape[-1]
        Lc = (L + 15) // 16 * 16
        out = np.zeros((128, Lc // 16), np.int16)
        for g in range(N_CORES):
            a = np.zeros(Lc, np.int16)
            a[:L] = arr[c, g]
            out[16 * g:16 * g + 16, :] = a.reshape(Lc // 16, 16).T
        return out

    idx_w = [wrap(idx_all, c) for c in range(N_CORES)]

    # local_scatter idxs: [128, ncols], per-partition independent; replicate
    # group g's stream on its 16 partitions
    sidx_w = []
    for c in range(N_CORES):
        o = np.zeros((128, ncols), np.int16)
        for g in range(N_CORES):
            o[16 * g:16 * g + 16, :] = scat_all[c, g][None, :]
        sidx_w.append(o)

    deg_full = np.full(ntot, 1e30, np.float32)
    deg_full[:n_nodes] = (deg_in[:n_nodes] + 1.0).astype(np.float32)

    return instrs, total_slots, ncols, tstart, idx_w, sidx_w, deg_full


def _build(nw, total_slots, ncols, tstart, instrs, chunk):
    nt = nw // TCH
    nc = bacc.Bacc("TRN2", target_bir_lowering=False, debug=False,
                   num_devices=N_CORES)
    dt = mybir.dt.float32
    bt = mybir.dt.bfloat16

    xT_in = nc.dram_tensor("xT", [F_IN, nw], bt, kind="ExternalInput")
    deg_in = nc.dram_tensor("degv", [1, nw], dt, kind="ExternalInput")
    ones_in = nc.dram_tensor("ones16", [1, 16], dt, kind="ExternalInput")
    ident_in = nc.dram_tensor("ident", [16, 16], dt, kind="ExternalInput")
    idx_in = nc.dram_tensor("idxs", [128, (total_slots + 15) // 16],
                            mybir.dt.int16, kind="ExternalInput")
    ridx_in = nc.dram_tensor("ridxs", [128, ncols], mybir.dt.int16,
                             kind="ExternalInput")
    blk_in = nc.dram_tensor("blk", [128, 16], dt, kind="ExternalInput")
    w1_in = nc.dram_tensor("W1", [F_IN, H], bt, kind="ExternalInput")
    b1_in = nc.dram_tensor("b1", [H, 1], dt, kind="ExternalInput")
    w2_in = nc.dram_tensor("W2", [H, H], bt, kind="ExternalInput")
    b2_in = nc.dram_tensor("b2", [H, 1], dt, kind="ExternalInput")
    wc_in = nc.dram_tensor("Wc", [H, A], dt, kind="ExternalInput")
    bc_in = nc.dram_tensor("bc", [1, A], dt, kind="ExternalInput")
    out_t = nc.dram_tensor("out", [1, A], dt, kind="ExternalOutput")

    with tile.TileContext(nc) as tc:
        with (
            tc.tile_pool(name="persist", bufs=1) as pp,
            tc.tile_pool(name="tabp", bufs=1) as tabp,
            tc.tile_pool(name="partp", bufs=1) as partp,
            tc.tile_pool(name="msgp", bufs=2) as msgp,
            tc.tile_pool(name="alnp", bufs=2) as alnp,
            tc.tile_pool(name="smallp", bufs=2) as smallp,
            tc.tile_pool(name="psum", bufs=2, space="PSUM") as psp,
            tc.tile_pool(name="dram", bufs=1, space="DRAM") as dram,
        ):
            w1 = pp.tile([F_IN, H], bt)
            nc.sync.dma_start(w1[:], w1_in[:])
            w2 = pp.tile([H, H], bt)
            nc.sync.dma_start(w2[:], w2_in[:])
            wc = pp.tile([H, A], dt)
            nc.sync.dma_start(wc[:], wc_in[:])
            b1 = pp.tile([H, 1], dt)
            nc.sync.dma_start(b1[:], b1_in[:])
            b2 = pp.tile([H, 1], dt)
            nc.sync.dma_start(b2[:], b2_in[:])
            bcb = pp.tile([1, A], dt)
            nc.sync.dma_start(bcb[:], bc_in[:])
            blk = pp.tile([128, 16], dt)
            nc.sync.dma_start(blk[:], blk_in[:])
            blkb = pp.tile([128, 16], bt)
            nc.vector.tensor_copy(blkb[:], blk[:])
            ones16 = pp.tile([1, 16], dt)
            nc.sync.dma_start(ones16[:], ones_in[:])
            idxs = pp.tile([128, (total_slots + 15) // 16], mybir.dt.int16)
            nc.sync.dma_start(idxs[:], idx_in[:])
            sidxs = pp.tile([128, ncols], mybir.dt.int16)
            nc.sync.dma_start(sidxs[:], ridx_in[:])

            pooled = smallp.tile([16, 1], dt, tag="pool")
            dis_dram = dram.tile([1, nw], dt, tag="disd")
            for j in range(0, nw, 512):
                cw = min(512, nw - j)
                degc = smallp.tile([1, 512], dt, tag="degc")
                nc.sync.dma_start(degc[:, :cw], deg_in[:, j:j + cw])
                nc.vector.reciprocal(degc[:, :cw], degc[:, :cw])
                disc = smallp.tile([1, 512], dt, tag="disc")
                nc.scalar.activation(disc[:, :cw], degc[:, :cw],
                                     mybir.ActivationFunctionType.Sqrt)
                nc.sync.dma_start(dis_dram[:, j:j + cw], disc[:, :cw])

            # 3 AllGather segments per layer, chunk-aligned
            SS = [0, 3 * TCH, 5 * TCH, nw]
            seg_of = [0, 0, 0, 1, 1, 2, 2]
            seg_last = {2: 0, 4: 1, 6: 2}
            ag_in = {}
            ag_out = {}
            for layer in range(2):
                ag_in[layer] = []
                ag_out[layer] = []
                for s in range(3):
                    agi = dram.tile([16, SS[s + 1] - SS[s]], dt,
                                    tag=f"agin{layer}_{s}")
                    ago = dram.tile([128, SS[s + 1] - SS[s]], dt,
                                    tag=f"agout{layer}_{s}")
                    ag_in[layer].append(agi)
                    ag_out[layer].append(ago)

            def fire_ag(layer, s):
                nc.gpsimd.collective_compute(
                    "AllGather", mybir.AluOpType.bypass,
                    replica_groups=[list(range(N_CORES))],
                    ins=[ag_in[layer][s].opt()], outs=[ag_out[layer][s].opt()],
                )

            def write_agin(layer, j, cw, sc_ap):
                lo = j
                while lo < j + cw:
                    s = 0 if lo < SS[1] else (1 if lo < SS[2] else 2)
                    hi = min(j + cw, SS[s + 1])
                    nc.sync.dma_start(
                        ag_in[layer][s][:, lo - SS[s]:hi - SS[s]],
                        sc_ap[:, lo - j:hi - j])
                    lo = hi

            # chunk -> last gather instr covering it
            last_i = {}
            for i, (_, _, spans) in enumerate(instrs):
                for (_, _, _, t, _) in spans:
                    last_i[t] = i
            chunks_at = {}
            for t, i in last_i.items():
                chunks_at.setdefault(i, []).append(t)

            # layer-0 transform from x, firing AG segments as they complete
            sb = nc.enter_named_scope("sbuild0", False)
            for j in range(0, nw, 512):
                cw = min(512, nw - j)
                xc = smallp.tile([F_IN, 512], bt, tag="xc")
                nc.sync.dma_start(xc[:, :cw], xT_in[:, j:j + cw])
                ps = psp.tile([16, 512], dt, tag="mma")
                nc.tensor.matmul(ps[:, :cw], lhsT=w1[:], rhs=xc[:, :cw],
                                 start=True, stop=True)
                disc = smallp.tile([1, 512], dt, tag="disc")
                nc.sync.dma_start(disc[:, :cw], dis_dram[:, j:j + cw])
                psd = psp.tile([16, 512], dt, tag="mmd")
                nc.tensor.matmul(psd[:, :cw], lhsT=ones16[:],
                                 rhs=disc[:, :cw], start=True,
                                 stop=True)
                ts = smallp.tile([16, 512], dt, tag="sc")
                nc.vector.tensor_copy(ts[:, :cw], ps[:, :cw])
                sc = smallp.tile([16, 512], dt, tag="sct")
                nc.vector.tensor_mul(sc[:, :cw], ts[:, :cw], psd[:, :cw])
                write_agin(0, j, cw, sc)
                for s in range(3):
                    if j < SS[s + 1] <= j + cw:
                        fire_ag(0, s)
            nc.leave_named_scope("sbuild0", sb[0], False)

            for layer in range(2):
                bvec = b1 if layer == 0 else b2

                table = tabp.tile([128, nw + 1], dt, tag="table")
                for s in range(3):
                    nc.sync.dma_start(table[:, SS[s]:SS[s + 1]],
                                      ag_out[layer][s][:])
                nc.vector.memset(table[:, nw:nw + 1], 0.0)

                partial = partp.tile([128, ncols], bt, tag="partial")
                nc.vector.memset(partial[:], 0.0)

                gsc = nc.enter_named_scope(f"gather{layer}", False)
                for i, (soff, nidx, spans) in enumerate(instrs):
                    msg = msgp.tile([128, chunk], dt, tag="msg")
                    nc.gpsimd.ap_gather(
                        out_ap=msg[:, :nidx],
                        in_ap=table[:],
                        idxs_ap=idxs[:, soff // 16:(soff + nidx) // 16],
                        channels=128, num_elems=nw + 1, d=1, num_idxs=nidx,
                    )
                    for (roff, m, k, t, coff) in spans:
                        pslc = partial[:, coff:coff + m]
                        if k == 1:
                            nc.vector.tensor_copy(
                                pslc, msg[:, roff:roff + m])
                        else:
                            mv = msg[:, roff:roff + m * k].rearrange(
                                "p (n k) -> p n k", k=k)
                            with nc.allow_low_precision(
                                    reason="fp32 msgs reduced once to bf16 "
                                           "partial; rel tol 2e-2"):
                                nc.vector.tensor_reduce(
                                    pslc, mv, axis=mybir.AxisListType.X,
                                    op=mybir.AluOpType.add)

                    # realign chunks whose gathers just completed; overlaps
                    # PE/DVE/DMA consumers (and layer-1 transform) with the
                    # remaining gather stream
                    for t in sorted(chunks_at.get(i, [])):
                        j0 = t * TCH
                        s = seg_of[t]
                        c0, c1 = int(tstart[t]), int(tstart[t + 1])
                        aln = alnp.tile([128, TCH], bt, tag="aln")
                        nc.gpsimd.local_scatter(
                            aln[:], partial[:, c0:c1], sidxs[:, c0:c1],
                            channels=128, num_elems=TCH, num_idxs=c1 - c0,
                        )
                        sob = alnp.tile([16, TCH], dt, tag="sob")
                        nc.sync.dma_start(
                            sob[:],
                            ag_in[layer][s][:, j0 - SS[s]:j0 - SS[s] + TCH])
                        for j1 in range(0, TCH, 512):
                            j = j0 + j1
                            cw = min(512, TCH - j1)
                            ps = psp.tile([16, 512], dt, tag="mma")
                            nc.tensor.matmul(ps[:, :cw], lhsT=blkb[:],
                                             rhs=aln[:, j1:j1 + cw],
                                             start=True, stop=True)
                            disc = smallp.tile([1, 512], dt, tag="disc")
                            nc.sync.dma_start(disc[:, :cw],
                                              dis_dram[:, j:j + cw])
                            psd = psp.tile([16, 512], dt, tag="mmd")
                            nc.tensor.matmul(psd[:, :cw], lhsT=ones16[:],
                                             rhs=disc[:, :cw],
                                             start=True, stop=True)
                            u = smallp.tile([16, 512], dt, tag="acc")
                            nc.vector.tensor_add(u[:, :cw],
                                                 sob[:, j1:j1 + cw],
                                                 ps[:, :cw])
                            v = smallp.tile([16, 512], dt, tag="tso")
                            nc.vector.tensor_mul(v[:, :cw], u[:, :cw],
                                                 psd[:, :cw])
                            if layer == 0:
                                hnb = smallp.tile([16, 512], bt, tag="hnb")
                                nc.scalar.activation(
                                    hnb[:, :cw], v[:, :cw],
                                    mybir.ActivationFunctionType.Relu,
                                    bias=bvec[:])
                                ps1 = psp.tile([16, 512], dt, tag="mm1")
                                nc.tensor.matmul(ps1[:, :cw], lhsT=w2[:],
                                                 rhs=hnb[:, :cw],
                                                 start=True, stop=True)
                                t1 = smallp.tile([16, 512], dt, tag="ts1")
                                nc.vector.tensor_copy(t1[:, :cw],
                                                      ps1[:, :cw])
                                s1 = smallp.tile([16, 512], dt, tag="sc1")
                                nc.vector.tensor_mul(s1[:, :cw], t1[:, :cw],
                                                     psd[:, :cw])
                                write_agin(1, j, cw, s1)
                            else:
                                hn = smallp.tile([16, 512], dt, tag="hn")
                                nc.scalar.activation(
                                    hn[:, :cw], v[:, :cw],
                                    mybir.ActivationFunctionType.Relu,
                                    bias=bvec[:])
                                pmx = smallp.tile([16, 1], dt, tag="pmx")
                                nc.vector.tensor_reduce(
                                    pmx[:], hn[:, :cw],
                                    axis=mybir.AxisListType.X,
                                    op=mybir.AluOpType.max)
                                if j == 0:
                                    nc.vector.tensor_copy(pooled[:], pmx[:])
                                else:
                                    nc.vector.tensor_tensor(
                                        pooled[:], pooled[:], pmx[:],
                                        op=mybir.AluOpType.max)
                        if layer == 0 and t in seg_last:
                            fire_ag(1, seg_last[t])
                nc.leave_named_scope(f"gather{layer}", gsc[0], False)

            pin = dram.tile([16, 1], dt, tag="pin")
            pout = dram.tile([16, 1], dt, tag="pout")
            nc.sync.dma_start(pin[:], pooled[:])
            nc.gpsimd.collective_compute(
                "AllReduce", mybir.AluOpType.max,
                replica_groups=[list(range(N_CORES))],
                ins=[pin.opt()], outs=[pout.opt()],
            )
            pooled2 = smallp.tile([16, 1], dt, tag="pool2")
            nc.sync.dma_start(pooled2[:], pout[:])
            ps = psp.tile([1, A], dt, tag="mmc")
            nc.tensor.matmul(ps[:], lhsT=pooled2[:], rhs=wc[:],
                             start=True, stop=True)
            ores = smallp.tile([1, A], dt, tag="ores")
            nc.vector.tensor_add(ores[:], ps[:], bcb[:])
            nc.sync.dma_start(out_t[:], ores[:])

    nc.compile()
    return nc


def _prepare(x, edge_index, W1, b1, W2, b2, Wc, bc, _nw=12544, _chunk=2048):
    n_nodes = x.shape[0]
    nw = _nw
    edge_index = np.asarray(edge_index)

    # chunk-balancing node relabel (pool output is permutation-invariant)
    old_of_new = _balance_perm(edge_index, n_nodes, nw)  # [N_CORES, nw]
    new_of_old = np.empty_like(old_of_new)
    for c in range(N_CORES):
        new_of_old[c, old_of_new[c]] = np.arange(nw)

    def relab(a):
        cc = a // nw
        return cc * nw + new_of_old[cc, a % nw]

    edge2 = np.stack([relab(edge_index[0].astype(np.int64)),
                      relab(edge_index[1].astype(np.int64))])

    instrs, total_slots, ncols, tstart, idx_w, sidx_w, _ = _plan(
        edge2, n_nodes, nw, _chunk)

    # degrees/dis in relabeled space; pad nodes (old id >= real count) inert
    deg_new = np.bincount(edge2[1], minlength=nw * N_CORES).astype(np.float64)
    deg_full = np.full(nw * N_CORES, 1e30, np.float32)
    for c in range(N_CORES):
        rc = min(max(n_nodes - c * nw, 0), nw)
        rn = old_of_new[c] < rc
        seg = deg_full[c * nw:(c + 1) * nw]
        seg[rn] = (deg_new[c * nw:(c + 1) * nw][rn] + 1.0).astype(np.float32)

    nc = _build(nw, total_slots, ncols, tstart, instrs, _chunk)

    x = np.asarray(x, np.float32)
    blk = np.zeros((128, 16), np.float32)
    blk[np.arange(128), np.arange(128) % 16] = 1.0

    in_maps = []
    for c in range(N_CORES):
        xw = np.zeros((nw, F_IN), np.float32)
        rc = min(max(n_nodes - c * nw, 0), nw)
        rn = old_of_new[c] < rc
        if rc > 0:
            xw[rn] = x[c * nw + old_of_new[c][rn]]
        import ml_dtypes
        in_maps.append({
            "xT": np.ascontiguousarray(xw.T).astype(ml_dtypes.bfloat16),
            "degv": deg_full[c * nw:(c + 1) * nw][None, :].copy(),
            "ones16": np.ones((1, 16), np.float32),
            "idxs": idx_w[c],
            "ridxs": sidx_w[c],
            "blk": blk,
            "ident": np.eye(16, dtype=np.float32),
            "W1": np.asarray(W1, np.float32).astype(ml_dtypes.bfloat16),
            "b1": np.asarray(b1, np.float32).reshape(H, 1),
            "W2": np.asarray(W2, np.float32).astype(ml_dtypes.bfloat16),
            "b2": np.asarray(b2, np.float32).reshape(H, 1),
            "Wc": np.asarray(Wc, np.float32),
            "bc": np.asarray(bc, np.float32).reshape(1, A),
        })
    return nc, in_maps


def kernel(x, edge_index, W1, b1, W2, b2, Wc, bc, _nw=12544, _chunk=2048,
           _run=None):
    nc, in_maps = _prepare(x, edge_index, W1, b1, W2, b2, Wc, bc,
                           _nw=_nw, _chunk=_chunk)

    if _run == "sim":
        from concourse.bass_interp import MultiCoreSim
        sim = MultiCoreSim(nc, num_cores=N_CORES, trace=False)
        for c in range(N_CORES):
            for k, v in in_maps[c].items():
                sim.cores[c].tensor(k)[:] = v
        sim.simulate()
        print("SIM time:", sim.global_time, "ns")
        return np.asarray(sim.cores[0].tensor("out")).reshape(A)

    res = bass_utils.run_bass_kernel_spmd(
        nc, in_maps, core_ids=list(range(N_CORES)),
        trace=bool(_run == "trace"))
    if _run == "trace":
        print("HW exec time:", res.exec_time_ns, "ns")
    return np.asarray(res.results[0]["out"]).reshape(A)

